# revision 34
# baseline (speedup 1.0000x reference)
"""AutoAggregation (sparse_attention) Trainium2 kernel.

Math (per row of length E=512):
  corr = irfft(fft(q) * conj(fft(k)))          -> also output (transposed)
  (w, d) = top6(corr); w = softmax(w)
  V[e] = sum_k w_k * v[(e + d_k) % E]
       = irfft(conj(rfft(h)) * rfft(v)),  h = sparse(w at d)

DFT-by-matmul: fp16 inputs (cast during DMA), fp16 forward DFT matrices,
float32r inverse; spectra f = 1..256 with the all-zero sin(f=256) column
repurposed as a DC (ones) accumulator, permuted to partition 0, plus an
odd-n correction matmul. Top-6 values via DVE max; h built as
(corr >= 6th_largest) * exp(corr - max)/Z  -- no scatter or indices needed.

Per block (512 rows): FRONT = load/transpose/forward q,k,v + cmul + corr;
BACK = top6/h/corrT-out + h forward + cmul + V. FRONT(t+1) is emitted
before BACK(t) so the PE always has next-block work during BACK's serial
chain.

Sharding: core c handles b == c (8 heads x 1024 rows = 8192 rows/core).
"""

import numpy as np

B, H, L, E = 8, 8, 1024, 512
NF = 256           # spectra f = 1..256
ROWS = H * L       # 8192 rows per core
BLK = 512          # rows per block
NBLK = ROWS // BLK
NRC = BLK // 128   # row-chunks per block
NEC = E // 128     # e-chunks
TOPK = 6

_cache = {}


def _consts():
    e = np.arange(E)
    f = np.arange(1, NF + 1)
    ang = 2 * np.pi * np.outer(e, f) / E
    C = np.cos(ang)
    S = np.sin(ang)
    # spare slot: sin col for f=256 is all zeros -> use it as the DC (ones)
    # accumulator. Permute f-tile 2 so that slot sits on partition 0.
    S[:, NF - 1] = 1.0
    perm = np.concatenate([np.arange(128), [NF - 1], np.arange(128, NF - 1)])
    C = C[:, perm]
    S = S[:, perm]
    fwd_cs16 = np.concatenate([C, S], axis=1).astype(np.float16)    # [512, 512]
    w = np.full(NF, 2.0)
    w[-1] = 1.0
    n = np.arange(E)
    angi = 2 * np.pi * np.outer(f, n) / E
    A = w[:, None] * np.cos(angi) / E
    Bm = -w[:, None] * np.sin(angi) / E
    A = A[perm, :]
    Bm = Bm[perm, :]
    inv_ab = np.concatenate([A, Bm], axis=1).astype(np.float32)     # [256, 1024]
    # odd-n correction: the f=256 slot of res_re carries Q256K256 + Q0K0 and
    # is multiplied by (-1)^n/E; the DC term needs +1/E, so add 2/E on odd n.
    odd = np.zeros((1, E), np.float32)
    odd[0, 1::2] = 2.0 / E
    return {
        "fwd_cs16": fwd_cs16,
        "inv_ab": inv_ab,
        "ident16": np.eye(128, dtype=np.float16),
        "odd_mask": odd,
    }


def _build():
    import concourse.bacc as bacc
    import concourse.mybir as mybir
    import concourse.tile as tile

    dt = mybir.dt
    f32, f32r, f16 = dt.float32, dt.float32r, dt.float16
    Alu = mybir.AluOpType
    Act = mybir.ActivationFunctionType

    nc = bacc.Bacc("TRN2", target_bir_lowering=False, debug=False, num_devices=8)

    qd = nc.dram_tensor("q", [ROWS, E], f32, kind="ExternalInput").ap()
    kd = nc.dram_tensor("k", [ROWS, E], f32, kind="ExternalInput").ap()
    vd = nc.dram_tensor("v", [ROWS, E], f32, kind="ExternalInput").ap()
    cs16_d = nc.dram_tensor("fwd_cs16", [E, 2 * NF], f16, kind="ExternalInput").ap()
    inv_d = nc.dram_tensor("inv_ab", [NF, 2 * E], f32, kind="ExternalInput").ap()
    id16_d = nc.dram_tensor("ident16", [128, 128], f16, kind="ExternalInput").ap()
    odd_d = nc.dram_tensor("odd_mask", [1, E], f32, kind="ExternalInput").ap()
    v_out = nc.dram_tensor("v_out", [ROWS, E], f16, kind="ExternalOutput").ap()
    c_out = nc.dram_tensor("corr_out", [E, ROWS], f16, kind="ExternalOutput").ap()

    with tile.TileContext(nc, num_cores=8) as tc:
        with (
            tc.tile_pool(name="consts", bufs=1) as pc,
            tc.tile_pool(name="pin", bufs=8) as pin,
            tc.tile_pool(name="xt", bufs=3) as pxt,
            tc.tile_pool(name="sp", bufs=4) as psp,
            tc.tile_pool(name="spv", bufs=12) as pspv,
            tc.tile_pool(name="rtmp", bufs=4) as prtmp,
            tc.tile_pool(name="res", bufs=3) as pres,
            tc.tile_pool(name="big", bufs=12) as pbig,
            tc.tile_pool(name="out", bufs=4) as pout,
            tc.tile_pool(name="h", bufs=6) as ph,
            tc.tile_pool(name="sm", bufs=8) as psm,
            tc.tile_pool(name="dcp", bufs=2) as pdc,
            tc.tile_pool(name="ppt", bufs=2, space="PSUM") as ppt,
            tc.tile_pool(name="ppf", bufs=4, space="PSUM") as ppf,
            tc.tile_pool(name="ppi", bufs=2, space="PSUM") as ppi,
        ):
            # ---- constants ----
            cs16_sb = pc.tile([128, NEC, 2 * NF], f16)
            nc.sync.dma_start(cs16_sb[:], cs16_d.rearrange("(c p) f -> p c f", p=128))
            inv_sb = pc.tile([128, 2, 2 * E], f32)
            nc.sync.dma_start(inv_sb[:], inv_d.rearrange("(c p) n -> p c n", p=128))
            id16_sb = pc.tile([128, 128], f16)
            nc.sync.dma_start(id16_sb[:], id16_d)
            odd_sb = pc.tile([1, E], f32)
            nc.sync.dma_start(odd_sb[:], odd_d)
            inv_r = pc.tile([128, 2, 2 * E], f32r)
            nc.vector.tensor_copy(inv_r[:], inv_sb[:])
            odd_r = pc.tile([1, E], f32r)
            nc.vector.tensor_copy(odd_r[:], odd_sb[:])

            def fwd_w16(ec, trig, ft):
                o = trig * NF + ft * 128
                return cs16_sb[:, ec, o:o + 128]

            def inv_w(fc, ab):
                return inv_r[:, fc, ab * E:ab * E + E]

            def transpose_pack2(src_getter, sb_pool, sb_tag):
                """4x4 [128,128] fp16 transposes; two e-tiles share one PSUM
                bank and one evac. Returns 4 views [128, BLK] (per e-tile)."""
                views = []
                for half in range(2):
                    ps = ppt.tile([128, 2 * BLK], f16, tag="ppt2")
                    for j in range(2):
                        for c in range(4):
                            nc.tensor.transpose(
                                ps[:, j * BLK + c * 128: j * BLK + (c + 1) * 128],
                                src_getter(c, half * 2 + j), id16_sb[:])
                    sb = sb_pool.tile([128, 2 * BLK], f16, tag=sb_tag)
                    if half == 0:
                        nc.scalar.copy(sb[:], ps[:])
                    else:
                        nc.vector.tensor_copy(sb[:], ps[:])
                    views.extend([sb[:, 0:BLK], sb[:, BLK:2 * BLK]])
                return views

            def forward(xT):
                """16 accumulating fp16 MMs -> 4 PSUM tiles [c0, c1, s0, s1]."""
                out = []
                for trig in range(2):
                    for ft in range(2):
                        psf = ppf.tile([128, BLK], f32, tag="ppf")
                        for ec in range(NEC):
                            nc.tensor.matmul(
                                psf[:], fwd_w16(ec, trig, ft), xT[ec][:],
                                start=(ec == 0), stop=(ec == NEC - 1))
                        out.append(psf)
                return out

            def cmul(a, b, tagpfx):
                """a: 4 SBUF tiles; b: 4 PSUM tiles. re = ac*bc + as*bs,
                im = ac*bs - as*bc. Mults on DVE, adds on GpSimd."""
                rr, ri = [], []
                for ft in range(2):
                    ac, as_ = a[ft], a[2 + ft]
                    bc, bs = b[ft], b[2 + ft]
                    t1 = prtmp.tile([128, BLK], f32, tag="tmpA")
                    nc.vector.tensor_tensor(t1[:], ac[:], bc[:], op=Alu.mult)
                    t2 = prtmp.tile([128, BLK], f32, tag="tmpB")
                    nc.vector.tensor_tensor(t2[:], as_[:], bs[:], op=Alu.mult)
                    re = pres.tile([128, BLK], f32r, tag=f"{tagpfx}_re")
                    nc.vector.tensor_tensor(re[:], t1[:], t2[:], op=Alu.add)
                    t3 = prtmp.tile([128, BLK], f32, tag="tmpA")
                    nc.vector.tensor_tensor(t3[:], ac[:], bs[:], op=Alu.mult)
                    t4 = prtmp.tile([128, BLK], f32, tag="tmpB")
                    nc.vector.tensor_tensor(t4[:], as_[:], bc[:], op=Alu.mult)
                    im = pres.tile([128, BLK], f32r, tag=f"{tagpfx}_im")
                    nc.gpsimd.tensor_tensor(im[:], t3[:], t4[:], op=Alu.subtract)
                    rr.append(re)
                    ri.append(im)
                return rr, ri

            def load_block(t):
                r0 = t * BLK
                tiles = {}
                for name, src in (("q", qd), ("k", kd), ("v", vd)):
                    xin = pin.tile([128, NEC, E], f16, tag="in")
                    nc.gpsimd.dma_start(
                        xin[:],
                        src[r0:r0 + BLK, :].rearrange("(c p) e -> p c e", p=128))
                    tiles[name] = xin
                return tiles

            def front(t, cur):
                """transposes/forwards q,k,v; cmul; inverse -> corr (f16)."""
                spec = {}
                for name in ("q", "k", "v"):
                    xin = cur[name]
                    xT = transpose_pack2(
                        lambda c, et: xin[:, c, et * 128:(et + 1) * 128],
                        pxt, f"xt_{name}")
                    fw = forward(xT)
                    if name == "k":
                        spec["k"] = fw      # stays in PSUM, consumed by cmul
                    else:
                        pool = psp if name == "q" else pspv
                        ev = []
                        for psf in fw:
                            sb = pool.tile([128, BLK], f32, tag=f"sp_{name}")
                            nc.scalar.copy(sb[:], psf[:])
                            ev.append(sb)
                        spec[name] = ev

                # DC products from the ones-column slot (partition 0 of s1)
                dcp = pdc.tile([1, BLK], f32r, tag="dc_prod")
                nc.vector.tensor_tensor(dcp[:], spec["q"][3][0:1, :].bitcast(f32),
                                        spec["k"][3][0:1, :], op=Alu.mult)

                rr1, ri1 = cmul(spec["q"], spec["k"], "r1")

                corr_sb = []
                for rc in range(NRC):
                    cs = slice(rc * 128, (rc + 1) * 128)
                    psC = ppi.tile([128, E], f32, tag="ppi")
                    nc.tensor.matmul(psC[:], rr1[0][:, cs], inv_w(0, 0),
                                     start=True, stop=False)
                    nc.tensor.matmul(psC[:], rr1[1][:, cs], inv_w(1, 0),
                                     start=False, stop=False)
                    nc.tensor.matmul(psC[:], ri1[0][:, cs], inv_w(0, 1),
                                     start=False, stop=False)
                    nc.tensor.matmul(psC[:], ri1[1][:, cs], inv_w(1, 1),
                                     start=False, stop=False)
                    nc.tensor.matmul(psC[:], dcp[0:1, cs], odd_r[:],
                                     start=False, stop=True)
                    csb = pbig.tile([128, E], f16, tag="corr")
                    nc.scalar.copy(csb[:], psC[:])
                    corr_sb.append(csb)
                return {"corr": corr_sb, "spv": spec["v"], "t": t}

            def back(st):
                t = st["t"]
                r0 = t * BLK
                corr_sb = st["corr"]
                spec_v = st["spv"]

                # ---- h chain (all fp16) ----
                h16s = []
                for rc in range(NRC):
                    csb = corr_sb[rc]
                    mx = psm.tile([128, 8], f32, tag="mx")
                    nc.vector.max(mx[:], csb[:])
                    ex6 = psm.tile([128, TOPK], f32, tag="ex6")
                    nc.vector.tensor_scalar(ex6[:], mx[:, 0:TOPK], mx[:, 0:1],
                                            None, op0=Alu.subtract)
                    nc.scalar.activation(ex6[:], ex6[:], Act.Exp)
                    zs = psm.tile([128, 1], f32, tag="zs")
                    nc.vector.reduce_sum(zs[:], ex6[:], axis=mybir.AxisListType.X)
                    rz = psm.tile([128, 1], f32, tag="rz")
                    nc.vector.reciprocal(rz[:], zs[:])
                    nmx = psm.tile([128, 1], f16, tag="nmx")
                    nc.vector.tensor_scalar(nmx[:], mx[:, 0:1], -1.0, None,
                                            op0=Alu.mult)
                    e1 = prtmp.tile([128, E], f16, tag="tmpE")
                    nc.scalar.activation(e1[:], csb[:], Act.Exp,
                                         bias=nmx[:], scale=1.0)
                    hm = prtmp.tile([128, E], f16, tag="tmpH")
                    nc.vector.scalar_tensor_tensor(
                        hm[:], csb[:], mx[:, TOPK - 1:TOPK], e1[:],
                        op0=Alu.is_ge, op1=Alu.mult)
                    h16 = ph.tile([128, E], f16, tag="h16")
                    nc.vector.tensor_scalar(h16[:], hm[:], rz[:], None,
                                            op0=Alu.mult)
                    h16s.append(h16)

                # ---- corr transpose out (fp16 packed) ----
                for half in range(2):
                    ps = ppt.tile([128, 2 * BLK], f16, tag="ppt2")
                    for j in range(2):
                        for c in range(4):
                            nc.tensor.transpose(
                                ps[:, j * BLK + c * 128: j * BLK + (c + 1) * 128],
                                corr_sb[c][:, (half * 2 + j) * 128:
                                            (half * 2 + j + 1) * 128],
                                id16_sb[:])
                    ct = pout.tile([128, 2 * BLK], f16, tag="corrT")
                    nc.scalar.copy(ct[:], ps[:])
                    for j in range(2):
                        et = half * 2 + j
                        nc.sync.dma_start(
                            c_out[et * 128:(et + 1) * 128, r0:r0 + BLK],
                            ct[:, j * BLK:(j + 1) * BLK])

                # ---- h forward + cmul + inverse -> V ----
                hT = transpose_pack2(
                    lambda c, et: h16s[c][:, et * 128:(et + 1) * 128], ph, "hT")
                spec_h = forward(hT)
                rr2, ri2 = cmul(spec_v, spec_h, "r2")
                dcv2 = pdc.tile([1, BLK], f32r, tag="dc_v2")
                nc.vector.tensor_tensor(dcv2[:], spec_v[3][0:1, :].bitcast(f32),
                                        spec_h[3][0:1, :], op=Alu.mult)
                for rc in range(NRC):
                    cs = slice(rc * 128, (rc + 1) * 128)
                    psV = ppi.tile([128, E], f32, tag="ppi")
                    nc.tensor.matmul(psV[:], rr2[0][:, cs], inv_w(0, 0),
                                     start=True, stop=False)
                    nc.tensor.matmul(psV[:], rr2[1][:, cs], inv_w(1, 0),
                                     start=False, stop=False)
                    nc.tensor.matmul(psV[:], ri2[0][:, cs], inv_w(0, 1),
                                     start=False, stop=False)
                    nc.tensor.matmul(psV[:], ri2[1][:, cs], inv_w(1, 1),
                                     start=False, stop=False)
                    nc.tensor.matmul(psV[:], dcv2[0:1, cs], odd_r[:],
                                     start=False, stop=True)
                    vsb = pout.tile([128, E], f16, tag="vout")
                    nc.scalar.copy(vsb[:], psV[:])
                    nc.sync.dma_start(
                        v_out[r0 + rc * 128:r0 + (rc + 1) * 128, :], vsb[:])

            # software pipeline, 3 deep: FRONT(t+2) emitted before BACK(t)
            loaded = load_block(0)
            pend = []
            for t in range(NBLK):
                cur = loaded
                if t + 1 < NBLK:
                    loaded = load_block(t + 1)
                pend.append(front(t, cur))
                if len(pend) > 2:
                    back(pend.pop(0))
            for st in pend:
                back(st)

    nc.compile()
    return nc


def _get_nc():
    if "nc" not in _cache:
        _cache["nc"] = _build()
        _cache["consts"] = _consts()
    return _cache["nc"], _cache["consts"]


def _run(inputs, trace=False):
    from concourse import bass_utils
    nc, consts = _get_nc()
    q = np.ascontiguousarray(np.asarray(inputs["queries"], dtype=np.float32))
    k = np.ascontiguousarray(np.asarray(inputs["keys"], dtype=np.float32))
    v = np.ascontiguousarray(np.asarray(inputs["values"], dtype=np.float32))
    in_maps = []
    for c in range(8):
        m = {
            "q": q[c].reshape(ROWS, E),
            "k": k[c].reshape(ROWS, E),
            "v": v[c].reshape(ROWS, E),
        }
        m.update(consts)
        in_maps.append(m)
    res = bass_utils.run_bass_kernel_spmd(
        nc, in_maps, core_ids=list(range(8)), trace=trace)
    V = np.stack([r["v_out"].reshape(H, L, E) for r in res.results]).astype(np.float32)
    C = np.stack([r["corr_out"].reshape(E, H, L) for r in res.results]).astype(np.float32)
    return (V, C), res


def kernel(**inputs):
    (V, C), _ = _run(inputs, trace=False)
    return V, C


# revision 35
# speedup vs baseline: 1.0238x; 1.0238x over previous
"""AutoAggregation (sparse_attention) Trainium2 kernel.

Math (per row of length E=512):
  corr = irfft(fft(q) * conj(fft(k)))          -> also output (transposed)
  (w, d) = top6(corr); w = softmax(w)
  V[e] = sum_k w_k * v[(e + d_k) % E]
       = irfft(conj(rfft(h)) * rfft(v)),  h = sparse(w at d)

DFT-by-matmul: fp16 inputs (cast during DMA), fp16 forward DFT matrices,
float32r inverse; spectra f = 1..256 with the all-zero sin(f=256) column
repurposed as a DC (ones) accumulator, permuted to partition 0, plus an
odd-n correction matmul. Top-6 values via DVE max; h built as
(corr >= 6th_largest) * exp(corr - max)/Z  -- no scatter or indices needed.

Per block (512 rows): FRONT = load/transpose/forward q,k,v + cmul + corr;
BACK = top6/h/corrT-out + h forward + cmul + V. FRONT(t+1) is emitted
before BACK(t) so the PE always has next-block work during BACK's serial
chain.

Sharding: core c handles b == c (8 heads x 1024 rows = 8192 rows/core).
"""

import numpy as np

B, H, L, E = 8, 8, 1024, 512
NF = 256           # spectra f = 1..256
ROWS = H * L       # 8192 rows per core
BLK = 512          # rows per block
NBLK = ROWS // BLK
NRC = BLK // 128   # row-chunks per block
NEC = E // 128     # e-chunks
TOPK = 6

_cache = {}


def _consts():
    e = np.arange(E)
    f = np.arange(1, NF + 1)
    ang = 2 * np.pi * np.outer(e, f) / E
    C = np.cos(ang)
    S = np.sin(ang)
    # spare slot: sin col for f=256 is all zeros -> use it as the DC (ones)
    # accumulator. Permute f-tile 2 so that slot sits on partition 0.
    S[:, NF - 1] = 1.0
    perm = np.concatenate([np.arange(128), [NF - 1], np.arange(128, NF - 1)])
    C = C[:, perm]
    S = S[:, perm]
    fwd_cs16 = np.concatenate([C, S], axis=1).astype(np.float16)    # [512, 512]
    w = np.full(NF, 2.0)
    w[-1] = 1.0
    n = np.arange(E)
    angi = 2 * np.pi * np.outer(f, n) / E
    A = w[:, None] * np.cos(angi) / E
    Bm = -w[:, None] * np.sin(angi) / E
    A = A[perm, :]
    Bm = Bm[perm, :]
    inv_ab = np.concatenate([A, Bm], axis=1).astype(np.float32)     # [256, 1024]
    # odd-n correction: the f=256 slot of res_re carries Q256K256 + Q0K0 and
    # is multiplied by (-1)^n/E; the DC term needs +1/E, so add 2/E on odd n.
    odd = np.zeros((1, E), np.float32)
    odd[0, 1::2] = 2.0 / E
    return {
        "fwd_cs16": fwd_cs16,
        "inv_ab": inv_ab,
        "ident16": np.eye(128, dtype=np.float16),
        "odd_mask": odd,
    }


def _build():
    import concourse.bacc as bacc
    import concourse.mybir as mybir
    import concourse.tile as tile

    dt = mybir.dt
    f32, f32r, f16 = dt.float32, dt.float32r, dt.float16
    Alu = mybir.AluOpType
    Act = mybir.ActivationFunctionType

    nc = bacc.Bacc("TRN2", target_bir_lowering=False, debug=False, num_devices=8)

    qd = nc.dram_tensor("q", [ROWS, E], f32, kind="ExternalInput").ap()
    kd = nc.dram_tensor("k", [ROWS, E], f32, kind="ExternalInput").ap()
    vd = nc.dram_tensor("v", [ROWS, E], f32, kind="ExternalInput").ap()
    cs16_d = nc.dram_tensor("fwd_cs16", [E, 2 * NF], f16, kind="ExternalInput").ap()
    inv_d = nc.dram_tensor("inv_ab", [NF, 2 * E], f32, kind="ExternalInput").ap()
    id16_d = nc.dram_tensor("ident16", [128, 128], f16, kind="ExternalInput").ap()
    odd_d = nc.dram_tensor("odd_mask", [1, E], f32, kind="ExternalInput").ap()
    v_out = nc.dram_tensor("v_out", [ROWS, E], f16, kind="ExternalOutput").ap()
    c_out = nc.dram_tensor("corr_out", [E, ROWS], f16, kind="ExternalOutput").ap()

    with tile.TileContext(nc, num_cores=8) as tc:
        with (
            tc.tile_pool(name="consts", bufs=1) as pc,
            tc.tile_pool(name="pin", bufs=8) as pin,
            tc.tile_pool(name="xt", bufs=3) as pxt,
            tc.tile_pool(name="sp", bufs=4) as psp,
            tc.tile_pool(name="spv", bufs=12) as pspv,
            tc.tile_pool(name="rtmp", bufs=4) as prtmp,
            tc.tile_pool(name="res", bufs=3) as pres,
            tc.tile_pool(name="big", bufs=12) as pbig,
            tc.tile_pool(name="out", bufs=4) as pout,
            tc.tile_pool(name="h", bufs=6) as ph,
            tc.tile_pool(name="sm", bufs=8) as psm,
            tc.tile_pool(name="dcp", bufs=2) as pdc,
            tc.tile_pool(name="ppt", bufs=2, space="PSUM") as ppt,
            tc.tile_pool(name="ppf", bufs=4, space="PSUM") as ppf,
            tc.tile_pool(name="ppi", bufs=2, space="PSUM") as ppi,
        ):
            # ---- constants ----
            cs16_sb = pc.tile([128, NEC, 2 * NF], f16)
            nc.sync.dma_start(cs16_sb[:], cs16_d.rearrange("(c p) f -> p c f", p=128))
            inv_sb = pc.tile([128, 2, 2 * E], f32)
            nc.sync.dma_start(inv_sb[:], inv_d.rearrange("(c p) n -> p c n", p=128))
            id16_sb = pc.tile([128, 128], f16)
            nc.sync.dma_start(id16_sb[:], id16_d)
            odd_sb = pc.tile([1, E], f32)
            nc.sync.dma_start(odd_sb[:], odd_d)
            inv_r = pc.tile([128, 2, 2 * E], f32r)
            nc.vector.tensor_copy(inv_r[:], inv_sb[:])
            odd_r = pc.tile([1, E], f32r)
            nc.vector.tensor_copy(odd_r[:], odd_sb[:])

            def fwd_w16(ec, trig, ft):
                o = trig * NF + ft * 128
                return cs16_sb[:, ec, o:o + 128]

            def inv_w(fc, ab):
                return inv_r[:, fc, ab * E:ab * E + E]

            def transpose_pack2(src_getter, sb_pool, sb_tag):
                """4x4 [128,128] fp16 transposes; two e-tiles share one PSUM
                bank and one evac. Returns 4 views [128, BLK] (per e-tile)."""
                views = []
                for half in range(2):
                    ps = ppt.tile([128, 2 * BLK], f16, tag="ppt2")
                    for j in range(2):
                        for c in range(4):
                            nc.tensor.transpose(
                                ps[:, j * BLK + c * 128: j * BLK + (c + 1) * 128],
                                src_getter(c, half * 2 + j), id16_sb[:])
                    sb = sb_pool.tile([128, 2 * BLK], f16, tag=sb_tag)
                    if half == 0:
                        nc.scalar.copy(sb[:], ps[:])
                    else:
                        nc.vector.tensor_copy(sb[:], ps[:])
                    views.extend([sb[:, 0:BLK], sb[:, BLK:2 * BLK]])
                return views

            def forward(xT):
                """16 accumulating fp16 MMs -> 4 PSUM tiles [c0, c1, s0, s1]."""
                out = []
                for trig in range(2):
                    for ft in range(2):
                        psf = ppf.tile([128, BLK], f32, tag="ppf")
                        for ec in range(NEC):
                            nc.tensor.matmul(
                                psf[:], fwd_w16(ec, trig, ft), xT[ec][:],
                                start=(ec == 0), stop=(ec == NEC - 1))
                        out.append(psf)
                return out

            def cmul(a, b, tagpfx):
                """a: 4 SBUF tiles; b: 4 PSUM tiles. re = ac*bc + as*bs,
                im = ac*bs - as*bc. Mults on DVE, adds on GpSimd."""
                rr, ri = [], []
                for ft in range(2):
                    ac, as_ = a[ft], a[2 + ft]
                    bc, bs = b[ft], b[2 + ft]
                    t1 = prtmp.tile([128, BLK], f32, tag="tmpA")
                    nc.vector.tensor_tensor(t1[:], ac[:], bc[:], op=Alu.mult)
                    t2 = prtmp.tile([128, BLK], f32, tag="tmpB")
                    nc.vector.tensor_tensor(t2[:], as_[:], bs[:], op=Alu.mult)
                    re = pres.tile([128, BLK], f32r, tag=f"{tagpfx}_re")
                    nc.vector.tensor_tensor(re[:], t1[:], t2[:], op=Alu.add)
                    t3 = prtmp.tile([128, BLK], f32, tag="tmpA")
                    nc.vector.tensor_tensor(t3[:], ac[:], bs[:], op=Alu.mult)
                    t4 = prtmp.tile([128, BLK], f32, tag="tmpB")
                    nc.vector.tensor_tensor(t4[:], as_[:], bc[:], op=Alu.mult)
                    im = pres.tile([128, BLK], f32r, tag=f"{tagpfx}_im")
                    nc.gpsimd.tensor_tensor(im[:], t3[:], t4[:], op=Alu.subtract)
                    rr.append(re)
                    ri.append(im)
                return rr, ri

            def load_block(t):
                r0 = t * BLK
                tiles = {}
                for name, src in (("q", qd), ("k", kd), ("v", vd)):
                    xin = pin.tile([128, NEC, E], f16, tag="in")
                    nc.gpsimd.dma_start(
                        xin[:],
                        src[r0:r0 + BLK, :].rearrange("(c p) e -> p c e", p=128))
                    tiles[name] = xin
                return tiles

            def front(t, cur):
                """transposes/forwards q,k,v; cmul; inverse -> corr (f16)."""
                spec = {}
                for name in ("q", "k", "v"):
                    xin = cur[name]
                    xT = transpose_pack2(
                        lambda c, et: xin[:, c, et * 128:(et + 1) * 128],
                        pxt, f"xt_{name}")
                    fw = forward(xT)
                    if name == "k":
                        spec["k"] = fw      # stays in PSUM, consumed by cmul
                    else:
                        pool = psp if name == "q" else pspv
                        ev = []
                        for psf in fw:
                            sb = pool.tile([128, BLK], f32, tag=f"sp_{name}")
                            nc.scalar.copy(sb[:], psf[:])
                            ev.append(sb)
                        spec[name] = ev

                # DC products from the ones-column slot (partition 0 of s1)
                dcp = pdc.tile([1, BLK], f32r, tag="dc_prod")
                nc.vector.tensor_tensor(dcp[:], spec["q"][3][0:1, :].bitcast(f32),
                                        spec["k"][3][0:1, :], op=Alu.mult)

                rr1, ri1 = cmul(spec["q"], spec["k"], "r1")

                corr_sb = []
                for rc in range(NRC):
                    cs = slice(rc * 128, (rc + 1) * 128)
                    psC = ppi.tile([128, E], f32, tag="ppi")
                    nc.tensor.matmul(psC[:], rr1[0][:, cs], inv_w(0, 0),
                                     start=True, stop=False)
                    nc.tensor.matmul(psC[:], rr1[1][:, cs], inv_w(1, 0),
                                     start=False, stop=False)
                    nc.tensor.matmul(psC[:], ri1[0][:, cs], inv_w(0, 1),
                                     start=False, stop=False)
                    nc.tensor.matmul(psC[:], ri1[1][:, cs], inv_w(1, 1),
                                     start=False, stop=False)
                    nc.tensor.matmul(psC[:], dcp[0:1, cs], odd_r[:],
                                     start=False, stop=True)
                    csb = pbig.tile([128, E], f16, tag="corr")
                    if rc % 2 == 0:
                        nc.scalar.copy(csb[:], psC[:])
                    else:
                        nc.vector.tensor_copy(csb[:], psC[:])
                    corr_sb.append(csb)
                return {"corr": corr_sb, "spv": spec["v"], "t": t}

            def back(st):
                t = st["t"]
                r0 = t * BLK
                corr_sb = st["corr"]
                spec_v = st["spv"]

                # ---- h chain (all fp16) ----
                h16s = []
                for rc in range(NRC):
                    csb = corr_sb[rc]
                    mx = psm.tile([128, 8], f32, tag="mx")
                    nc.vector.max(mx[:], csb[:])
                    ex6 = psm.tile([128, TOPK], f32, tag="ex6")
                    nc.vector.tensor_scalar(ex6[:], mx[:, 0:TOPK], mx[:, 0:1],
                                            None, op0=Alu.subtract)
                    nc.scalar.activation(ex6[:], ex6[:], Act.Exp)
                    zs = psm.tile([128, 1], f32, tag="zs")
                    nc.vector.reduce_sum(zs[:], ex6[:], axis=mybir.AxisListType.X)
                    rz = psm.tile([128, 1], f32, tag="rz")
                    nc.vector.reciprocal(rz[:], zs[:])
                    nmx = psm.tile([128, 1], f16, tag="nmx")
                    nc.vector.tensor_scalar(nmx[:], mx[:, 0:1], -1.0, None,
                                            op0=Alu.mult)
                    e1 = prtmp.tile([128, E], f16, tag="tmpE")
                    nc.scalar.activation(e1[:], csb[:], Act.Exp,
                                         bias=nmx[:], scale=1.0)
                    hm = prtmp.tile([128, E], f16, tag="tmpH")
                    nc.vector.scalar_tensor_tensor(
                        hm[:], csb[:], mx[:, TOPK - 1:TOPK], e1[:],
                        op0=Alu.is_ge, op1=Alu.mult)
                    h16 = ph.tile([128, E], f16, tag="h16")
                    nc.vector.tensor_scalar(h16[:], hm[:], rz[:], None,
                                            op0=Alu.mult)
                    h16s.append(h16)

                # ---- corr transpose out (fp16 packed) ----
                for half in range(2):
                    ps = ppt.tile([128, 2 * BLK], f16, tag="ppt2")
                    for j in range(2):
                        for c in range(4):
                            nc.tensor.transpose(
                                ps[:, j * BLK + c * 128: j * BLK + (c + 1) * 128],
                                corr_sb[c][:, (half * 2 + j) * 128:
                                            (half * 2 + j + 1) * 128],
                                id16_sb[:])
                    ct = pout.tile([128, 2 * BLK], f16, tag="corrT")
                    nc.scalar.copy(ct[:], ps[:])
                    for j in range(2):
                        et = half * 2 + j
                        nc.sync.dma_start(
                            c_out[et * 128:(et + 1) * 128, r0:r0 + BLK],
                            ct[:, j * BLK:(j + 1) * BLK])

                # ---- h forward + cmul + inverse -> V ----
                hT = transpose_pack2(
                    lambda c, et: h16s[c][:, et * 128:(et + 1) * 128], ph, "hT")
                spec_h = forward(hT)
                rr2, ri2 = cmul(spec_v, spec_h, "r2")
                dcv2 = pdc.tile([1, BLK], f32r, tag="dc_v2")
                nc.vector.tensor_tensor(dcv2[:], spec_v[3][0:1, :].bitcast(f32),
                                        spec_h[3][0:1, :], op=Alu.mult)
                for rc in range(NRC):
                    cs = slice(rc * 128, (rc + 1) * 128)
                    psV = ppi.tile([128, E], f32, tag="ppi")
                    nc.tensor.matmul(psV[:], rr2[0][:, cs], inv_w(0, 0),
                                     start=True, stop=False)
                    nc.tensor.matmul(psV[:], rr2[1][:, cs], inv_w(1, 0),
                                     start=False, stop=False)
                    nc.tensor.matmul(psV[:], ri2[0][:, cs], inv_w(0, 1),
                                     start=False, stop=False)
                    nc.tensor.matmul(psV[:], ri2[1][:, cs], inv_w(1, 1),
                                     start=False, stop=False)
                    nc.tensor.matmul(psV[:], dcv2[0:1, cs], odd_r[:],
                                     start=False, stop=True)
                    vsb = pout.tile([128, E], f16, tag="vout")
                    if rc % 2 == 0:
                        nc.scalar.copy(vsb[:], psV[:])
                    else:
                        nc.vector.tensor_copy(vsb[:], psV[:])
                    nc.sync.dma_start(
                        v_out[r0 + rc * 128:r0 + (rc + 1) * 128, :], vsb[:])

            # software pipeline, 3 deep: FRONT(t+2) emitted before BACK(t)
            loaded = load_block(0)
            pend = []
            for t in range(NBLK):
                cur = loaded
                if t + 1 < NBLK:
                    loaded = load_block(t + 1)
                pend.append(front(t, cur))
                if len(pend) > 2:
                    back(pend.pop(0))
            for st in pend:
                back(st)

    nc.compile()
    return nc


def _get_nc():
    if "nc" not in _cache:
        _cache["nc"] = _build()
        _cache["consts"] = _consts()
    return _cache["nc"], _cache["consts"]


def _run(inputs, trace=False):
    from concourse import bass_utils
    nc, consts = _get_nc()
    q = np.ascontiguousarray(np.asarray(inputs["queries"], dtype=np.float32))
    k = np.ascontiguousarray(np.asarray(inputs["keys"], dtype=np.float32))
    v = np.ascontiguousarray(np.asarray(inputs["values"], dtype=np.float32))
    in_maps = []
    for c in range(8):
        m = {
            "q": q[c].reshape(ROWS, E),
            "k": k[c].reshape(ROWS, E),
            "v": v[c].reshape(ROWS, E),
        }
        m.update(consts)
        in_maps.append(m)
    res = bass_utils.run_bass_kernel_spmd(
        nc, in_maps, core_ids=list(range(8)), trace=trace)
    V = np.stack([r["v_out"].reshape(H, L, E) for r in res.results]).astype(np.float32)
    C = np.stack([r["corr_out"].reshape(E, H, L) for r in res.results]).astype(np.float32)
    return (V, C), res


def kernel(**inputs):
    (V, C), _ = _run(inputs, trace=False)
    return V, C
